# revision 20
# baseline (speedup 1.0000x reference)
"""Trainium2 Bass kernel: row-softmax + embedding gather (batched lookup).

reference:
    probs = softmax(poi_freq_matrix, axis=1)        # [100000, 168] f32
    out   = probs[inputs_wekn]                      # [1024, 200, 168] f32

Strategy (8 NeuronCores, data-parallel over batch; each core owns 128
batch rows = 128 SBUF partitions x 200 seq positions).

Gather: the table is packed into [50000, 2x192] bf16 PAIR rows (768B =
3x256, a legal SWDGE stride).  int16 indices only reach 32768 rows, but
the gather ucode's address math is IVP_MULUSAN_2X32 (unsigned stride x
SIGNED idx), so with the in_ap base advanced to pair 32768 the signed
idx = wekn//2 - 32768 in [-32768, 17231] addresses all 50000 pairs.
This halves gather traffic vs the 4-row-quad layout (768B vs 1536B per
lookup) and needs ONE predicated sub-row select instead of three.
The ucode trims TRAILING negative idxs, so each chunk's list gets one
extra slot-row of always-positive pad indices (gathers garbage pair
32768 into a scratch slot).

Emission: dma_gather descriptor emission runs on one Q7 CPU pair
selected by queue_num (~8ns/idx serial).  Gathers round-robin over all
4 SWDGE queues = 4 distinct CPU pairs; the Pool sequencer dispatches
ahead, so up to 4 emissions overlap (~2ns/idx effective).

Tail per chunk: one DVE copy_predicated picks the odd sub-row (uint32
pairs, 84/row); ACT exp -> bf16; DVE bf16 fold (168->84, 2x mode) +
tensor_reduce + reciprocal + one bulk tensor_tensor scale; bf16 HBM
store (f32 upcast on host).  GpSimd compute is avoided while DVE is
hot (they share an SBUF port).
"""

import sys

import numpy as np

sys.path.insert(0, "/opt/trn_rl_repo")

N_POI = 100000
N_BINS = 168
DP = 192  # padded row length in bf16 elems (384B)
NPAIR = N_POI // 2  # pair rows
IDX_BASE = 32768  # in_ap base offset; idx16 = pair_id - IDX_BASE
BATCH = 1024
SEQ = 200
N_CORES = 8
BPC = BATCH // N_CORES  # batch rows per core = 128 partitions

CHUNKS = (2, 5, 8, 11, 14, 20, 20, 20, 20, 20, 20, 20, 20)

_NC_CACHE = {}


def build(chunks=CHUNKS, nqueues=4, scratch=32768, tbufs=6, pbufs=5):
    """Build the per-core Bass program (SPMD: same NEFF on all cores)."""
    import concourse.bacc as bacc
    import concourse.tile as tile
    from concourse import bass, mybir

    seq = sum(chunks)
    mx = max(chunks)
    slots = seq + len(chunks)  # one pad slot-row per chunk
    nc = bacc.Bacc(
        "TRN2",
        target_bir_lowering=False,
        debug=False,
        enable_asserts=False,
        num_devices=N_CORES,
        num_swdge_queues=nqueues,
        dynamic_dma_scratch_size=scratch,
        enable_partition_id=False,
    )
    ptab = nc.dram_tensor(
        "ptab", [NPAIR, 2 * DP], mybir.dt.bfloat16, kind="ExternalInput"
    ).ap()
    widx = nc.dram_tensor(
        "widx", [128, slots * 8], mybir.dt.int16, kind="ExternalInput"
    ).ap()
    msk = nc.dram_tensor(
        "msk", [BPC, seq], mybir.dt.uint8, kind="ExternalInput"
    ).ap()
    out = nc.dram_tensor(
        "out", [BPC, seq, N_BINS], mybir.dt.bfloat16, kind="ExternalOutput"
    ).ap()

    with tile.TileContext(nc) as tc:
        with tc.tile_pool(name="const", bufs=1) as cpool, tc.tile_pool(
            name="quad", bufs=tbufs
        ) as tpool, tc.tile_pool(name="prob", bufs=pbufs) as ppool, tc.tile_pool(
            name="small", bufs=8
        ) as smpool, tc.tile_pool(name="foldp", bufs=4) as fpool:
            m0 = chunks[0]
            wt = cpool.tile([128, slots * 8], mybir.dt.int16)
            # chunk 0's idx slice loads first so its gather starts ASAP
            nc.sync.dma_start(out=wt[:, : (m0 + 1) * 8], in_=widx[:, : (m0 + 1) * 8])
            nc.sync.dma_start(out=wt[:, (m0 + 1) * 8 :], in_=widx[:, (m0 + 1) * 8 :])
            mt = cpool.tile([BPC, seq], mybir.dt.uint8)
            nc.sync.dma_start(out=mt[:], in_=msk[:])

            off = 0  # position offset (output side)
            soff = 0  # slot offset in the idx table (incl pads)
            for ci, m in enumerate(chunks):
                ms = m + 1  # slots incl the positive-pad row
                # full-size tiles (one tag each) sliced to this chunk
                Tf = tpool.tile([BPC, (mx + 1) * 2 * DP], mybir.dt.bfloat16, tag="T")
                T4 = Tf[:, : ms * 2 * DP].rearrange(
                    "p (m q d) -> p m q d", m=ms, q=2
                )
                nc.gpsimd.dma_gather(
                    out_ap=Tf[:, : ms * 2 * DP].rearrange(
                        "p (m d) -> p m d", m=ms
                    ),
                    in_ap=ptab[IDX_BASE:],
                    idxs_ap=wt[:, soff * 8 : (soff + ms) * 8],
                    num_idxs=BPC * ms,
                    num_idxs_reg=BPC * ms,
                    elem_size=2 * DP,
                    elem_step=2 * DP,
                    single_packet=False,
                    queue_num=ci % nqueues,
                )
                # one select: odd sub-row over even, as uint32 pairs
                Tu = Tf[:, : ms * 2 * DP].bitcast(mybir.dt.uint32)
                U4 = Tu.rearrange("p (m q d) -> p m q d", m=ms, q=2)
                nc.vector.copy_predicated(
                    out=U4[:, :m, 0, : N_BINS // 2],
                    mask=mt[:, off : off + m].to_broadcast(
                        [BPC, m, N_BINS // 2]
                    ),
                    data=U4[:, :m, 1, : N_BINS // 2],
                )
                sel = T4[:, :m, 0, :N_BINS]
                Pf = ppool.tile([BPC, mx * N_BINS], mybir.dt.bfloat16, tag="P")
                P3 = Pf[:, : m * N_BINS].rearrange("p (m d) -> p m d", m=m)
                nc.scalar.activation(
                    out=P3, in_=sel, func=mybir.ActivationFunctionType.Exp
                )
                # fold 168 -> 84 in bf16 (2x DVE mode) so the 1x
                # tensor_reduce walks half the elements
                fold = fpool.tile([BPC, mx * 84], mybir.dt.bfloat16, tag="fold")
                F3 = fold[:, : m * 84].rearrange("p (m d) -> p m d", m=m)
                nc.vector.tensor_tensor(
                    out=F3,
                    in0=P3[:, :, 0:84],
                    in1=P3[:, :, 84:168],
                    op=mybir.AluOpType.add,
                )
                sums = smpool.tile([BPC, mx], mybir.dt.float32, tag="sums")
                nc.vector.tensor_reduce(
                    out=sums[:, :m],
                    in_=F3,
                    axis=mybir.AxisListType.X,
                    op=mybir.AluOpType.add,
                )
                rec = smpool.tile([BPC, mx], mybir.dt.float32, tag="rec")
                nc.vector.reciprocal(out=rec[:, :m], in_=sums[:, :m])
                # bulk DVE scale (one tensor_tensor per chunk)
                nc.vector.tensor_tensor(
                    out=P3,
                    in0=P3,
                    in1=rec[:, :m].to_broadcast([BPC, m, N_BINS]),
                    op=mybir.AluOpType.mult,
                )
                nc.sync.dma_start(
                    out=out[:, off : off + m, :], in_=Pf[:, : m * N_BINS]
                )
                off += m
                soff += ms
    nc.compile()
    return nc


def _prep_inputs(wekn, table, chunks=CHUNKS):
    """Host-side layout/index prep: bf16 cast, padded pair table, signed
    int16 pair ids (base-shifted), odd-row masks, per-core shards."""
    import ml_dtypes

    seq = sum(chunks)
    slots = seq + len(chunks)
    tb = table.astype(ml_dtypes.bfloat16)
    pt = np.zeros((NPAIR, 2, DP), dtype=ml_dtypes.bfloat16)
    pt[:, :, :N_BINS] = tb.reshape(NPAIR, 2, N_BINS)
    pt = np.ascontiguousarray(pt.reshape(NPAIR, 2 * DP))
    in_maps = []
    for core in range(N_CORES):
        wc = wekn[core * BPC : (core + 1) * BPC]
        pair = (wc // 2 - IDX_BASE).astype(np.int16)
        sub = (wc % 2).astype(np.uint8)
        wi = np.empty((16, slots * 8), dtype=np.int16)
        off = 0
        soff = 0
        for m in chunks:
            ms = m + 1
            walk = np.empty(ms * BPC, dtype=np.int16)
            walk[: m * BPC] = pair[:, off : off + m].T.reshape(-1)
            walk[m * BPC :] = 0  # positive pad row (prevents trailing-trim)
            wi[:, soff * 8 : (soff + ms) * 8] = walk.reshape(ms * 8, 16).T
            off += m
            soff += ms
        in_maps.append(
            {
                "ptab": pt,
                "widx": np.tile(wi, (8, 1)),
                "msk": np.ascontiguousarray(sub),
            }
        )
    return in_maps


def _get_nc():
    if "nc" not in _NC_CACHE:
        _NC_CACHE["nc"] = build()
    return _NC_CACHE["nc"]


def kernel(**inputs) -> np.ndarray:
    wekn = np.asarray(inputs["inputs_wekn"]).astype(np.int64)
    table = np.ascontiguousarray(
        np.asarray(inputs["poi_freq_matrix"], dtype=np.float32)
    )
    assert wekn.shape == (BATCH, SEQ) and table.shape == (N_POI, N_BINS)

    from concourse.bass_utils import run_bass_kernel_spmd

    nc = _get_nc()
    in_maps = _prep_inputs(wekn, table)
    res = run_bass_kernel_spmd(nc, in_maps, core_ids=list(range(N_CORES)))
    return np.concatenate(
        [np.asarray(res.results[c]["out"]).astype(np.float32) for c in range(N_CORES)],
        axis=0,
    )


if __name__ == "__main__":
    rng = np.random.default_rng(0)
    inputs = {
        "venueid2coor": rng.random((N_POI, 2), dtype=np.float32),
        "inputs_wekn": rng.integers(0, N_POI, size=(BATCH, SEQ), dtype=np.int64),
        "poi_freq_matrix": rng.standard_normal((N_POI, N_BINS), dtype=np.float32),
    }
    out = kernel(**inputs)
    print(out.shape, out.dtype)


# revision 21
# speedup vs baseline: 1.0847x; 1.0847x over previous
"""Trainium2 Bass kernel: row-softmax + embedding gather (batched lookup).

reference:
    probs = softmax(poi_freq_matrix, axis=1)        # [100000, 168] f32
    out   = probs[inputs_wekn]                      # [1024, 200, 168] f32

Strategy (8 NeuronCores, data-parallel over batch; each core owns 128
batch rows = 128 SBUF partitions x 200 seq positions).

Gather: the table is packed into [50000, 2x192] bf16 PAIR rows (768B =
3x256, a legal SWDGE stride).  int16 indices only reach 32768 rows, but
the gather ucode's address math is IVP_MULUSAN_2X32 (unsigned stride x
SIGNED idx), so with the in_ap base advanced to pair 32768 the signed
idx = wekn//2 - 32768 in [-32768, 17231] addresses all 50000 pairs.
This halves gather traffic vs the 4-row-quad layout (768B vs 1536B per
lookup) and needs ONE predicated sub-row select instead of three.
The ucode trims TRAILING negative idxs, so each chunk's list gets one
extra slot-row of always-positive pad indices (gathers garbage pair
32768 into a scratch slot).

Emission: dma_gather descriptor emission runs on one Q7 CPU pair
selected by queue_num (~8ns/idx serial).  Gathers round-robin over all
4 SWDGE queues = 4 distinct CPU pairs; the Pool sequencer dispatches
ahead, so up to 4 emissions overlap (~2ns/idx effective).

Tail per chunk: one DVE copy_predicated picks the odd sub-row (uint32
pairs, 84/row); ACT exp -> bf16; DVE bf16 fold (168->84, 2x mode) +
tensor_reduce + reciprocal + one bulk tensor_tensor scale; bf16 HBM
store (f32 upcast on host).  GpSimd compute is avoided while DVE is
hot (they share an SBUF port).
"""

import sys

import numpy as np

sys.path.insert(0, "/opt/trn_rl_repo")

N_POI = 100000
N_BINS = 168
DP = 192  # padded row length in bf16 elems (384B)
NPAIR = N_POI // 2  # pair rows
IDX_BASE = 32768  # in_ap base offset; idx16 = pair_id - IDX_BASE
BATCH = 1024
SEQ = 200
N_CORES = 8
BPC = BATCH // N_CORES  # batch rows per core = 128 partitions

CHUNKS = (2, 5, 8, 11, 13, 13, 13, 13, 13, 13, 13, 13, 13, 13, 13, 13, 13, 5)

_NC_CACHE = {}


def build(chunks=CHUNKS, nqueues=4, scratch=32768, tbufs=8, pbufs=6):
    """Build the per-core Bass program (SPMD: same NEFF on all cores)."""
    import concourse.bacc as bacc
    import concourse.tile as tile
    from concourse import bass, mybir

    seq = sum(chunks)
    mx = max(chunks)
    slots = seq + len(chunks)  # one pad slot-row per chunk
    nc = bacc.Bacc(
        "TRN2",
        target_bir_lowering=False,
        debug=False,
        enable_asserts=False,
        num_devices=N_CORES,
        num_swdge_queues=nqueues,
        dynamic_dma_scratch_size=scratch,
        enable_partition_id=False,
    )
    ptab = nc.dram_tensor(
        "ptab", [NPAIR, 2 * DP], mybir.dt.bfloat16, kind="ExternalInput"
    ).ap()
    widx = nc.dram_tensor(
        "widx", [128, slots * 8], mybir.dt.int16, kind="ExternalInput"
    ).ap()
    msk = nc.dram_tensor(
        "msk", [BPC, seq], mybir.dt.uint8, kind="ExternalInput"
    ).ap()
    out = nc.dram_tensor(
        "out", [BPC, seq, N_BINS], mybir.dt.bfloat16, kind="ExternalOutput"
    ).ap()

    with tile.TileContext(nc) as tc:
        with tc.tile_pool(name="const", bufs=1) as cpool, tc.tile_pool(
            name="quad", bufs=tbufs
        ) as tpool, tc.tile_pool(name="prob", bufs=pbufs) as ppool, tc.tile_pool(
            name="small", bufs=8
        ) as smpool, tc.tile_pool(name="foldp", bufs=4) as fpool:
            m0 = chunks[0]
            wt = cpool.tile([128, slots * 8], mybir.dt.int16)
            # chunk 0's idx slice loads first so its gather starts ASAP
            nc.sync.dma_start(out=wt[:, : (m0 + 1) * 8], in_=widx[:, : (m0 + 1) * 8])
            nc.sync.dma_start(out=wt[:, (m0 + 1) * 8 :], in_=widx[:, (m0 + 1) * 8 :])
            mt = cpool.tile([BPC, seq], mybir.dt.uint8)
            nc.sync.dma_start(out=mt[:], in_=msk[:])

            off = 0  # position offset (output side)
            soff = 0  # slot offset in the idx table (incl pads)
            for ci, m in enumerate(chunks):
                ms = m + 1  # slots incl the positive-pad row
                # full-size tiles (one tag each) sliced to this chunk
                Tf = tpool.tile([BPC, (mx + 1) * 2 * DP], mybir.dt.bfloat16, tag="T")
                T4 = Tf[:, : ms * 2 * DP].rearrange(
                    "p (m q d) -> p m q d", m=ms, q=2
                )
                nc.gpsimd.dma_gather(
                    out_ap=Tf[:, : ms * 2 * DP].rearrange(
                        "p (m d) -> p m d", m=ms
                    ),
                    in_ap=ptab[IDX_BASE:],
                    idxs_ap=wt[:, soff * 8 : (soff + ms) * 8],
                    num_idxs=BPC * ms,
                    num_idxs_reg=BPC * ms,
                    elem_size=2 * DP,
                    elem_step=2 * DP,
                    single_packet=False,
                    queue_num=ci % nqueues,
                )
                # one select: odd sub-row over even, as uint32 pairs
                Tu = Tf[:, : ms * 2 * DP].bitcast(mybir.dt.uint32)
                U4 = Tu.rearrange("p (m q d) -> p m q d", m=ms, q=2)
                nc.vector.copy_predicated(
                    out=U4[:, :m, 0, : N_BINS // 2],
                    mask=mt[:, off : off + m].to_broadcast(
                        [BPC, m, N_BINS // 2]
                    ),
                    data=U4[:, :m, 1, : N_BINS // 2],
                )
                sel = T4[:, :m, 0, :N_BINS]
                Pf = ppool.tile([BPC, mx * N_BINS], mybir.dt.bfloat16, tag="P")
                P3 = Pf[:, : m * N_BINS].rearrange("p (m d) -> p m d", m=m)
                nc.scalar.activation(
                    out=P3, in_=sel, func=mybir.ActivationFunctionType.Exp
                )
                # fold 168 -> 84 in bf16 (2x DVE mode) so the 1x
                # tensor_reduce walks half the elements
                fold = fpool.tile([BPC, mx * 84], mybir.dt.bfloat16, tag="fold")
                F3 = fold[:, : m * 84].rearrange("p (m d) -> p m d", m=m)
                nc.vector.tensor_tensor(
                    out=F3,
                    in0=P3[:, :, 0:84],
                    in1=P3[:, :, 84:168],
                    op=mybir.AluOpType.add,
                )
                sums = smpool.tile([BPC, mx], mybir.dt.float32, tag="sums")
                nc.vector.tensor_reduce(
                    out=sums[:, :m],
                    in_=F3,
                    axis=mybir.AxisListType.X,
                    op=mybir.AluOpType.add,
                )
                rec = smpool.tile([BPC, mx], mybir.dt.float32, tag="rec")
                nc.vector.reciprocal(out=rec[:, :m], in_=sums[:, :m])
                # bulk DVE scale (one tensor_tensor per chunk)
                nc.vector.tensor_tensor(
                    out=P3,
                    in0=P3,
                    in1=rec[:, :m].to_broadcast([BPC, m, N_BINS]),
                    op=mybir.AluOpType.mult,
                )
                nc.sync.dma_start(
                    out=out[:, off : off + m, :], in_=Pf[:, : m * N_BINS]
                )
                off += m
                soff += ms
    nc.compile()
    return nc


def _prep_inputs(wekn, table, chunks=CHUNKS):
    """Host-side layout/index prep: bf16 cast, padded pair table, signed
    int16 pair ids (base-shifted), odd-row masks, per-core shards."""
    import ml_dtypes

    seq = sum(chunks)
    slots = seq + len(chunks)
    tb = table.astype(ml_dtypes.bfloat16)
    pt = np.zeros((NPAIR, 2, DP), dtype=ml_dtypes.bfloat16)
    pt[:, :, :N_BINS] = tb.reshape(NPAIR, 2, N_BINS)
    pt = np.ascontiguousarray(pt.reshape(NPAIR, 2 * DP))
    in_maps = []
    for core in range(N_CORES):
        wc = wekn[core * BPC : (core + 1) * BPC]
        pair = (wc // 2 - IDX_BASE).astype(np.int16)
        sub = (wc % 2).astype(np.uint8)
        wi = np.empty((16, slots * 8), dtype=np.int16)
        off = 0
        soff = 0
        for m in chunks:
            ms = m + 1
            walk = np.empty(ms * BPC, dtype=np.int16)
            walk[: m * BPC] = pair[:, off : off + m].T.reshape(-1)
            walk[m * BPC :] = 0  # positive pad row (prevents trailing-trim)
            wi[:, soff * 8 : (soff + ms) * 8] = walk.reshape(ms * 8, 16).T
            off += m
            soff += ms
        in_maps.append(
            {
                "ptab": pt,
                "widx": np.tile(wi, (8, 1)),
                "msk": np.ascontiguousarray(sub),
            }
        )
    return in_maps


def _get_nc():
    if "nc" not in _NC_CACHE:
        _NC_CACHE["nc"] = build()
    return _NC_CACHE["nc"]


def kernel(**inputs) -> np.ndarray:
    wekn = np.asarray(inputs["inputs_wekn"]).astype(np.int64)
    table = np.ascontiguousarray(
        np.asarray(inputs["poi_freq_matrix"], dtype=np.float32)
    )
    assert wekn.shape == (BATCH, SEQ) and table.shape == (N_POI, N_BINS)

    from concourse.bass_utils import run_bass_kernel_spmd

    nc = _get_nc()
    in_maps = _prep_inputs(wekn, table)
    res = run_bass_kernel_spmd(nc, in_maps, core_ids=list(range(N_CORES)))
    return np.concatenate(
        [np.asarray(res.results[c]["out"]).astype(np.float32) for c in range(N_CORES)],
        axis=0,
    )


if __name__ == "__main__":
    rng = np.random.default_rng(0)
    inputs = {
        "venueid2coor": rng.random((N_POI, 2), dtype=np.float32),
        "inputs_wekn": rng.integers(0, N_POI, size=(BATCH, SEQ), dtype=np.int64),
        "poi_freq_matrix": rng.standard_normal((N_POI, N_BINS), dtype=np.float32),
    }
    out = kernel(**inputs)
    print(out.shape, out.dtype)


# revision 22
# speedup vs baseline: 1.1804x; 1.0883x over previous
"""Trainium2 Bass kernel: row-softmax + embedding gather (batched lookup).

reference:
    probs = softmax(poi_freq_matrix, axis=1)        # [100000, 168] f32
    out   = probs[inputs_wekn]                      # [1024, 200, 168] f32

Strategy (8 NeuronCores, data-parallel over batch; each core owns 128
batch rows = 128 SBUF partitions x 200 seq positions).

Gather: the table is packed into [50000, 2x192] bf16 PAIR rows (768B =
3x256, a legal SWDGE stride).  int16 indices only reach 32768 rows, but
the gather ucode's address math is IVP_MULUSAN_2X32 (unsigned stride x
SIGNED idx), so with the in_ap base advanced to pair 32768 the signed
idx = wekn//2 - 32768 in [-32768, 17231] addresses all 50000 pairs.
This halves gather traffic vs the 4-row-quad layout (768B vs 1536B per
lookup) and needs ONE predicated sub-row select instead of three.
The ucode trims TRAILING negative idxs, so each chunk's list gets one
extra slot-row of always-positive pad indices (gathers garbage pair
32768 into a scratch slot).

Emission: dma_gather descriptor emission runs on one Q7 CPU pair
selected by queue_num (~8ns/idx serial).  Gathers round-robin over all
4 SWDGE queues = 4 distinct CPU pairs; the Pool sequencer dispatches
ahead, so up to 4 emissions overlap (~2ns/idx effective).

Tail per chunk: one DVE copy_predicated picks the odd sub-row (uint32
pairs, 84/row); ACT exp -> bf16; DVE bf16 fold (168->84, 2x mode) +
tensor_reduce + reciprocal + one bulk tensor_tensor scale; bf16 HBM
store (f32 upcast on host).  GpSimd compute is avoided while DVE is
hot (they share an SBUF port).
"""

import sys

import numpy as np

sys.path.insert(0, "/opt/trn_rl_repo")

N_POI = 100000
N_BINS = 168
DP = 192  # padded row length in bf16 elems (384B)
NPAIR = N_POI // 2  # pair rows
IDX_BASE = 32768  # in_ap base offset; idx16 = pair_id - IDX_BASE
BATCH = 1024
SEQ = 200
N_CORES = 8
BPC = BATCH // N_CORES  # batch rows per core = 128 partitions

CHUNKS = (2, 5, 8, 11, 13, 13, 13, 13, 13, 13, 13, 13, 13, 13, 13, 13, 13, 5)

_NC_CACHE = {}


def build(chunks=CHUNKS, nqueues=4, scratch=32768, tbufs=8, pbufs=6):
    """Build the per-core Bass program (SPMD: same NEFF on all cores)."""
    import concourse.bacc as bacc
    import concourse.tile as tile
    from concourse import bass, mybir

    seq = sum(chunks)
    mx = max(chunks)
    slots = seq + len(chunks)  # one pad slot-row per chunk
    nc = bacc.Bacc(
        "TRN2",
        target_bir_lowering=False,
        debug=False,
        enable_asserts=False,
        num_devices=N_CORES,
        num_swdge_queues=nqueues,
        dynamic_dma_scratch_size=scratch,
        enable_partition_id=False,
    )
    ptab = nc.dram_tensor(
        "ptab", [NPAIR, 2 * DP], mybir.dt.bfloat16, kind="ExternalInput"
    ).ap()
    widx = nc.dram_tensor(
        "widx", [128, slots * 8], mybir.dt.int16, kind="ExternalInput"
    ).ap()
    msk = nc.dram_tensor(
        "msk", [BPC, seq], mybir.dt.uint8, kind="ExternalInput"
    ).ap()
    out = nc.dram_tensor(
        "out", [BPC, seq, N_BINS], mybir.dt.bfloat16, kind="ExternalOutput"
    ).ap()

    with tile.TileContext(nc) as tc:
        with tc.tile_pool(name="const", bufs=1) as cpool, tc.tile_pool(
            name="quad", bufs=tbufs
        ) as tpool, tc.tile_pool(name="prob", bufs=pbufs) as ppool, tc.tile_pool(
            name="small", bufs=8
        ) as smpool, tc.tile_pool(name="foldp", bufs=4) as fpool:
            m0 = chunks[0]
            wt = cpool.tile([128, slots * 8], mybir.dt.int16)
            # chunk 0's idx slice loads first so its gather starts ASAP
            nc.sync.dma_start(out=wt[:, : (m0 + 1) * 8], in_=widx[:, : (m0 + 1) * 8])
            nc.sync.dma_start(out=wt[:, (m0 + 1) * 8 :], in_=widx[:, (m0 + 1) * 8 :])
            mt = cpool.tile([BPC, seq], mybir.dt.uint8)
            nc.sync.dma_start(out=mt[:], in_=msk[:])

            off = 0  # position offset (output side)
            soff = 0  # slot offset in the idx table (incl pads)
            for ci, m in enumerate(chunks):
                ms = m + 1  # slots incl the positive-pad row
                # full-size tiles (one tag each) sliced to this chunk
                Tf = tpool.tile([BPC, (mx + 1) * 2 * DP], mybir.dt.bfloat16, tag="T")
                T4 = Tf[:, : ms * 2 * DP].rearrange(
                    "p (m q d) -> p m q d", m=ms, q=2
                )
                nc.gpsimd.dma_gather(
                    out_ap=Tf[:, : ms * 2 * DP].rearrange(
                        "p (m d) -> p m d", m=ms
                    ),
                    in_ap=ptab[IDX_BASE:],
                    idxs_ap=wt[:, soff * 8 : (soff + ms) * 8],
                    num_idxs=BPC * ms,
                    num_idxs_reg=BPC * ms,
                    elem_size=2 * DP,
                    elem_step=2 * DP,
                    single_packet=False,
                    queue_num=ci % nqueues,
                )
                # one select: odd sub-row over even, as uint32 pairs
                Tu = Tf[:, : ms * 2 * DP].bitcast(mybir.dt.uint32)
                U4 = Tu.rearrange("p (m q d) -> p m q d", m=ms, q=2)
                nc.vector.copy_predicated(
                    out=U4[:, :m, 0, : N_BINS // 2],
                    mask=mt[:, off : off + m].to_broadcast(
                        [BPC, m, N_BINS // 2]
                    ),
                    data=U4[:, :m, 1, : N_BINS // 2],
                )
                sel = T4[:, :m, 0, :N_BINS]
                Pf = ppool.tile([BPC, mx * N_BINS], mybir.dt.bfloat16, tag="P")
                P3 = Pf[:, : m * N_BINS].rearrange("p (m d) -> p m d", m=m)
                nc.scalar.activation(
                    out=P3, in_=sel, func=mybir.ActivationFunctionType.Exp
                )
                # fold 168 -> 84 in bf16 (2x DVE mode) so the 1x
                # tensor_reduce walks half the elements
                fold = fpool.tile([BPC, mx * 84], mybir.dt.bfloat16, tag="fold")
                F3 = fold[:, : m * 84].rearrange("p (m d) -> p m d", m=m)
                nc.vector.tensor_tensor(
                    out=F3,
                    in0=P3[:, :, 0:84],
                    in1=P3[:, :, 84:168],
                    op=mybir.AluOpType.add,
                )
                sums = smpool.tile([BPC, mx], mybir.dt.float32, tag="sums")
                nc.vector.tensor_reduce(
                    out=sums[:, :m],
                    in_=F3,
                    axis=mybir.AxisListType.X,
                    op=mybir.AluOpType.add,
                )
                rec = smpool.tile([BPC, mx], mybir.dt.float32, tag="rec")
                nc.vector.reciprocal(out=rec[:, :m], in_=sums[:, :m])
                # ACT (idle) materializes the broadcast recip row in bf16 so
                # the DVE scale runs as two 2x-mode TTs (a step-0 broadcast
                # operand would lock tensor_tensor to 1x)
                recb = fpool.tile([BPC, mx * 84], mybir.dt.bfloat16, tag="recb")
                R3 = recb[:, : m * 84].rearrange("p (m d) -> p m d", m=m)
                nc.scalar.activation(
                    out=R3,
                    in_=rec[:, :m].to_broadcast([BPC, m, 84]),
                    func=mybir.ActivationFunctionType.Copy,
                )
                nc.vector.tensor_tensor(
                    out=P3[:, :, 0:84],
                    in0=P3[:, :, 0:84],
                    in1=R3,
                    op=mybir.AluOpType.mult,
                )
                nc.vector.tensor_tensor(
                    out=P3[:, :, 84:168],
                    in0=P3[:, :, 84:168],
                    in1=R3,
                    op=mybir.AluOpType.mult,
                )
                nc.sync.dma_start(
                    out=out[:, off : off + m, :], in_=Pf[:, : m * N_BINS]
                )
                off += m
                soff += ms
    nc.compile()
    return nc


def _prep_inputs(wekn, table, chunks=CHUNKS):
    """Host-side layout/index prep: bf16 cast, padded pair table, signed
    int16 pair ids (base-shifted), odd-row masks, per-core shards."""
    import ml_dtypes

    seq = sum(chunks)
    slots = seq + len(chunks)
    tb = table.astype(ml_dtypes.bfloat16)
    pt = np.zeros((NPAIR, 2, DP), dtype=ml_dtypes.bfloat16)
    pt[:, :, :N_BINS] = tb.reshape(NPAIR, 2, N_BINS)
    pt = np.ascontiguousarray(pt.reshape(NPAIR, 2 * DP))
    in_maps = []
    for core in range(N_CORES):
        wc = wekn[core * BPC : (core + 1) * BPC]
        pair = (wc // 2 - IDX_BASE).astype(np.int16)
        sub = (wc % 2).astype(np.uint8)
        wi = np.empty((16, slots * 8), dtype=np.int16)
        off = 0
        soff = 0
        for m in chunks:
            ms = m + 1
            walk = np.empty(ms * BPC, dtype=np.int16)
            walk[: m * BPC] = pair[:, off : off + m].T.reshape(-1)
            walk[m * BPC :] = 0  # positive pad row (prevents trailing-trim)
            wi[:, soff * 8 : (soff + ms) * 8] = walk.reshape(ms * 8, 16).T
            off += m
            soff += ms
        in_maps.append(
            {
                "ptab": pt,
                "widx": np.tile(wi, (8, 1)),
                "msk": np.ascontiguousarray(sub),
            }
        )
    return in_maps


def _get_nc():
    if "nc" not in _NC_CACHE:
        _NC_CACHE["nc"] = build()
    return _NC_CACHE["nc"]


def kernel(**inputs) -> np.ndarray:
    wekn = np.asarray(inputs["inputs_wekn"]).astype(np.int64)
    table = np.ascontiguousarray(
        np.asarray(inputs["poi_freq_matrix"], dtype=np.float32)
    )
    assert wekn.shape == (BATCH, SEQ) and table.shape == (N_POI, N_BINS)

    from concourse.bass_utils import run_bass_kernel_spmd

    nc = _get_nc()
    in_maps = _prep_inputs(wekn, table)
    res = run_bass_kernel_spmd(nc, in_maps, core_ids=list(range(N_CORES)))
    return np.concatenate(
        [np.asarray(res.results[c]["out"]).astype(np.float32) for c in range(N_CORES)],
        axis=0,
    )


if __name__ == "__main__":
    rng = np.random.default_rng(0)
    inputs = {
        "venueid2coor": rng.random((N_POI, 2), dtype=np.float32),
        "inputs_wekn": rng.integers(0, N_POI, size=(BATCH, SEQ), dtype=np.int64),
        "poi_freq_matrix": rng.standard_normal((N_POI, N_BINS), dtype=np.float32),
    }
    out = kernel(**inputs)
    print(out.shape, out.dtype)
